# revision 1
# baseline (speedup 1.0000x reference)
"""Multi-head self-attention (RoPE, causal) Bass kernel for 8 TRN2 NeuronCores.

Sharding: tensor-parallel over heads for QKV+attention (2 heads/core),
chunked AllToAll (2 chunks/unit, fp16) overlapped with attention compute,
then token-parallel O-projection (strided 64-token strips per core).

All matmul operands are fp16 (FWL weight loads, halved DMA/DVE traffic);
PSUM accumulation stays fp32. exp uses bias=-4 so fp16 attention weights
cannot overflow; the bias cancels in the softmax normalization.

Layouts (per core):
  qT/kT: [128 part = 2 heads x 64 dk, t] fp16 (projection outputs + RoPE)
  sc:    [128 part = k-tile, 2 heads x 512 q] PSUM f32
  at:    same shape, fp16, exp(sc*0.125 - 4), causal-masked via 0/1 TT mult
  v_sb:  [128 part = k-tile tokens, 16 kt, 130] fp16 ([v_h0 | 1 | v_h1 | 1])
  outT_h:[65 part = 64 dk + denom row, 512 q] PSUM f32 per head
  aoT:   [128, t] fp16 normalized attention output (PE outer-product recip
         broadcast, no DRAM round-trip)
  y:     [128 part = 2 x 64-token strips, 1024] f32 per (unit, qi-pair)
"""

import numpy as np

B, S, D, H, DK = 2, 2048, 1024, 16, 64
NC = 8
THETA = 10000.0

_COMPILED = {}


def _build():
    import concourse.bass as bass
    import concourse.tile as tile
    from concourse import bacc, mybir

    f32 = mybir.dt.float32
    f32r = mybir.dt.float32r
    f16 = mybir.dt.float16
    MUL = mybir.AluOpType.mult
    EXP = mybir.ActivationFunctionType.Exp

    nc = bacc.Bacc(num_devices=NC)

    xh_d = nc.dram_tensor("xh", [B, 4, 128, 8, 512], f16, kind="ExternalInput")
    wqkv_d = nc.dram_tensor("wqkv", [128, 3, 8, 128], f16, kind="ExternalInput")
    wo_d = nc.dram_tensor("wo", [128, 8, 1024], f16, kind="ExternalInput")
    cs_d = nc.dram_tensor("cs", [128, 2, S], f16, kind="ExternalInput")
    cb_d = nc.dram_tensor("cb", [128, 672], f16, kind="ExternalInput")
    onesr_d = nc.dram_tensor("onesr", [1, 64], f32r, kind="ExternalInput")
    y_d = nc.dram_tensor("y", [B, 2, 128, 1024], f32, kind="ExternalOutput")

    SWAP_MASK = [(i ^ 1) for i in range(32)]
    EBIAS_VAL = -4.0

    with tile.TileContext(nc) as tc:
        with (
            tc.tile_pool(name="const", bufs=1) as constp,
            tc.tile_pool(name="xtp", bufs=2) as xtp,
            tc.tile_pool(name="qk", bufs=1) as qkp,
            tc.tile_pool(name="vp", bufs=1) as vp,
            tc.tile_pool(name="attn", bufs=3) as attnp,
            tc.tile_pool(name="ao", bufs=2) as aop,
            tc.tile_pool(name="rtmp", bufs=2) as rtmp,
            tc.tile_pool(name="recipp", bufs=2) as recipp,
            tc.tile_pool(name="rbp", bufs=2) as rbp,
            tc.tile_pool(name="gp", bufs=2) as gp,
            tc.tile_pool(name="yp", bufs=2) as yp,
            tc.tile_pool(name="ps", bufs=4, space="PSUM") as psp,
            tc.tile_pool(name="dram", bufs=1, space="DRAM") as dramp,
        ):
            # ---- constant tiles ----
            wqkv_sb = constp.tile([128, 3, 8, 128], f16)
            wo_sb = constp.tile([128, 8, 1024], f16)
            cs_sb = constp.tile([128, 2, S], f16)
            cb_sb = constp.tile([128, 672], f16)
            onesr_sb = constp.tile([1, 64], f32r)

            cost = cs_sb[:, 0, :]
            sinmt = cs_sb[:, 1, :]
            mask0 = cb_sb[:, 0:256]
            mask1 = cb_sb[:, 256:512]
            identb = cb_sb[:, 512:640]
            onesb = cb_sb[:, 640:656]
            ebias = cb_sb[:, 656:657]

            # critical path: projection weights first (sync queue)
            nc.sync.dma_start(wqkv_sb[:], wqkv_d[:])

            warm_in = dramp.tile([NC, 64], f32, name="warm_in")
            warm_out = dramp.tile([NC, 64], f32, name="warm_out")
            nc.gpsimd.collective_compute(
                "AllToAll",
                mybir.AluOpType.bypass,
                replica_groups=[list(range(NC))],
                ins=[warm_in.opt()],
                outs=[warm_out.opt()],
            )
            nc.gpsimd.dma_start(cs_sb[:], cs_d[:])
            nc.gpsimd.dma_start(wo_sb[:], wo_d[:])

            a2ain = [
                [dramp.tile([NC, 128, 64], f16, name=f"a2ai{u}_{c}") for c in range(4)]
                for u in range(B)
            ]
            a2aout = [
                [dramp.tile([NC, 128, 64], f16, name=f"a2ao{u}_{c}") for c in range(4)]
                for u in range(B)
            ]

            def o_projection(u, pair):
                g = gp.tile([128, 8, 128], f16, tag="g", name="g")
                for half in range(2):
                    nc.sync.dma_start(
                        g[:, :, half * 64:(half + 1) * 64],
                        a2aout[u][pair * 2 + half].rearrange("j p t -> p j t"),
                    )
                y_ps = psp.tile([128, 1024], f32, tag="ps", name="y_ps")
                for src in range(8):
                    for os_ in range(2):
                        nc.tensor.matmul(
                            y_ps[:, os_ * 512:(os_ + 1) * 512],
                            g[:, src, :],
                            wo_sb[:, src, os_ * 512:(os_ + 1) * 512],
                            start=(src == 0), stop=(src == 7),
                        )
                y_sb = yp.tile([128, 1024], f32, tag="y", name="y_sb")
                nc.vector.tensor_copy(out=y_sb[:], in_=y_ps[:])
                nc.gpsimd.dma_start(y_d[u, pair], y_sb[:])

            # deferred normalize+ship state: (u, qi, outh0, outh1, aoT)
            pending = []

            def emit_normalize():
                u, qi, outT, aoT = pending.pop()
                qsl = slice(qi * 512, (qi + 1) * 512)
                dentr = recipp.tile([1, 1024], f32r, tag="recip", name="dentr")
                nc.vector.tensor_copy(out=dentr[:], in_=outT[64:65, :])
                rb_ps = psp.tile([128, 1024], f32, tag="ps", name="rb_ps")
                for h in (0, 1):
                    # broadcast denominators to 64 partitions via an
                    # f32r ones-column outer product (both halves at
                    # partition base 0 -- tile_position (0,64) fails the
                    # walrus ISA check)
                    nc.tensor.matmul(
                        rb_ps[0:64, h * 512:(h + 1) * 512],
                        onesr_sb[:],
                        dentr[0:1, h * 512:(h + 1) * 512],
                        start=True, stop=True,
                    )
                rb_sb = rbp.tile([128, 1024], f32, tag="rb", name="rb_sb")
                nc.vector.reciprocal_approx_fast(out=rb_sb[0:64, :], in_=rb_ps[0:64, :])
                for h in (0, 1):
                    nc.vector.tensor_tensor(
                        out=aoT[h * 64:(h + 1) * 64, qsl],
                        in0=outT[0:64, h * 512:(h + 1) * 512],
                        in1=rb_sb[0:64, h * 512:(h + 1) * 512],
                        op=MUL,
                    )
                # ship this q-slab as its own a2a chunk
                dst = a2ain[u][qi].rearrange("j p t -> p j t")
                src = aoT[:, qsl].rearrange("p (j t) -> p j t", j=8)
                nc.gpsimd.dma_start(dst, src)
                nc.gpsimd.collective_compute(
                    "AllToAll",
                    mybir.AluOpType.bypass,
                    replica_groups=[list(range(NC))],
                    ins=[a2ain[u][qi].opt()],
                    outs=[a2aout[u][qi].opt()],
                )

            for u in range(B):
                # ================= projections + RoPE =================
                qT = qkp.tile([128, S], f16, tag="qT", name="qT")
                kT = qkp.tile([128, S], f16, tag="kT", name="kT")
                v_sb = vp.tile([128, 16, 130], f16, tag="v", name="v_sb")

                for tt in range(4):
                    ts = slice(tt * 512, (tt + 1) * 512)
                    xt_sb = xtp.tile([128, 8, 512], f16, tag="xt", name="xt_sb")
                    nc.sync.dma_start(xt_sb[:], xh_d[u, tt])
                    if u == 0 and tt == 0:
                        nc.sync.dma_start(cb_sb[:], cb_d[:])
                        nc.sync.dma_start(onesr_sb[:], onesr_d[:])
                    qk_ps = psp.tile([128, 1024], f32, tag="ps", name="qk_ps")
                    v_ps = psp.tile([128, 1024], f32, tag="ps", name="v_ps")
                    for dc in range(8):
                        st = dc == 0
                        sp = dc == 7
                        rhs = xt_sb[:, dc, :]
                        nc.tensor.matmul(qk_ps[:, 0:512], wqkv_sb[:, 0, dc, :], rhs, start=st, stop=sp)
                        nc.tensor.matmul(qk_ps[:, 512:1024], wqkv_sb[:, 1, dc, :], rhs, start=st, stop=sp)
                        nc.tensor.matmul(v_ps[:, 0:512], wqkv_sb[:, 2, dc, :], rhs, start=st, stop=sp)
                    if tt == 0 and u > 0:
                        # prior unit's first O-projection (chunk A landed long
                        # ago; emitted BEFORE the ccB trigger below so it
                        # does not wait on the cumulative cc semaphore), then
                        # the prior unit's qi3 normalize + chunk-B collective.
                        o_projection(u - 1, 0)
                        while pending:
                            emit_normalize()

                    # RoPE: dst = q*cos + pairswap(q)*sinm
                    for src, dst in ((qk_ps[:, 0:512], qT), (qk_ps[:, 512:1024], kT)):
                        qs = rtmp.tile([128, 512], f32, tag="qs", name="qs")
                        t2 = rtmp.tile([128, 512], f16, tag="t2", name="t2")
                        nc.vector.stream_shuffle(qs[:], src, SWAP_MASK)
                        nc.vector.tensor_tensor(out=dst[:, ts], in0=src, in1=cost[:, ts], op=MUL)
                        nc.vector.tensor_tensor(out=t2[:], in0=qs[:], in1=sinmt[:, ts], op=MUL)
                        nc.vector.tensor_tensor(out=dst[:, ts], in0=dst[:, ts], in1=t2[:], op=mybir.AluOpType.add)

                    # v -> token-major via PE transpose (fp16)
                    vtmp = rtmp.tile([128, 512], f16, tag="vtmp", name="vtmp")
                    nc.vector.tensor_copy(out=vtmp[:], in_=v_ps[:, 0:512])
                    vtr = v_ps[:, 512:1024].bitcast(f16)
                    for s4 in range(4):
                        kt = tt * 4 + s4
                        tr = vtr[:, s4 * 128:(s4 + 1) * 128]
                        nc.tensor.transpose(tr, vtmp[:, s4 * 128:(s4 + 1) * 128], identb)
                        dstv = v_sb[:, kt, :].rearrange("p (u c) -> p u c", u=2)[:, :, 0:64]
                        srcv = tr.rearrange("p (u c) -> p u c", u=2)
                        nc.vector.tensor_copy(out=dstv, in_=srcv)
                    nc.vector.tensor_copy(out=v_sb[:, tt * 4:(tt + 1) * 4, 64:65],
                                          in_=onesb[:, tt * 4:(tt + 1) * 4])
                    nc.vector.tensor_copy(out=v_sb[:, tt * 4:(tt + 1) * 4, 129:130],
                                          in_=onesb[:, tt * 4:(tt + 1) * 4])

                # ================= attention (software-pipelined) =================
                aoT = aop.tile([128, S], f16, tag="aoT", name="aoT")
                for qi in range(4):
                    qsl = slice(qi * 512, (qi + 1) * 512)
                    outT = psp.tile([128, 1024], f32, tag="ps", name="outT")
                    n_kt = 4 * qi + 4
                    prev = None  # (kt, at, diag_pos)

                    def emit_attnv(kt, at, dp):
                        for h in (0, 1):
                            lhs = v_sb[:, kt, :].rearrange("p (u c) -> p u c", u=2)[:, h, :]
                            if dp < 2:
                                nc.tensor.matmul(
                                    outT[0:65, h * 512:(h + 1) * 512],
                                    lhs,
                                    at[:, h * 512:(h + 1) * 512],
                                    start=(kt == 0), stop=(kt == n_kt - 1),
                                    skip_group_check=True,
                                )
                            else:
                                nc.tensor.matmul(
                                    outT[0:65, h * 512 + 256:h * 512 + 512],
                                    lhs,
                                    at[:, h * 512 + 256:h * 512 + 512],
                                    start=False, stop=(kt == n_kt - 1),
                                    skip_group_check=True,
                                )

                    for kt in range(n_kt):
                        ksl = slice(kt * 128, (kt + 1) * 128)
                        dp = kt - 4 * qi
                        sc = psp.tile([128, 1024], f32, tag="ps", name="sc")
                        at = attnp.tile([128, 1024], f16, tag="at", name="at")
                        if dp < 2:
                            for h in (0, 1):
                                hp = slice(h * 64, (h + 1) * 64)
                                nc.tensor.matmul(
                                    sc[:, h * 512:(h + 1) * 512],
                                    kT[hp, ksl],
                                    qT[hp, qsl],
                                    start=True, stop=True,
                                    skip_group_check=True,
                                )
                            nc.scalar.activation(out=at[:], in_=sc[:], func=EXP,
                                                 scale=0.125)
                            if dp == 0:
                                for h in (0, 1):
                                    o = h * 512
                                    nc.vector.tensor_tensor(
                                        out=at[:, o:o + 128], in0=at[:, o:o + 128],
                                        in1=mask0[:, 0:128], op=MUL)
                            elif dp == 1:
                                for h in (0, 1):
                                    o = h * 512
                                    nc.vector.tensor_tensor(
                                        out=at[:, o:o + 256], in0=at[:, o:o + 256],
                                        in1=mask1[:, 0:256], op=MUL)
                        else:
                            # kt2/kt3 of the diagonal: only q columns 256:512
                            for h in (0, 1):
                                hp = slice(h * 64, (h + 1) * 64)
                                o = h * 512
                                nc.tensor.matmul(
                                    sc[:, o + 256:o + 512],
                                    kT[hp, ksl],
                                    qT[hp, qsl][:, 256:512],
                                    start=True, stop=True,
                                    skip_group_check=True,
                                )
                                nc.scalar.activation(
                                    out=at[:, o + 256:o + 512], in_=sc[:, o + 256:o + 512],
                                    func=EXP, scale=0.125)
                            if dp == 2:
                                for h in (0, 1):
                                    o = h * 512
                                    nc.vector.tensor_tensor(
                                        out=at[:, o + 256:o + 384], in0=at[:, o + 256:o + 384],
                                        in1=mask0[:, 0:128], op=MUL)
                            else:
                                for h in (0, 1):
                                    o = h * 512
                                    nc.vector.tensor_tensor(
                                        out=at[:, o + 256:o + 512], in0=at[:, o + 256:o + 512],
                                        in1=mask1[:, 0:256], op=MUL)

                        if prev is not None:
                            emit_attnv(*prev)
                        if kt == 1 and pending:
                            emit_normalize()
                        prev = (kt, at, dp)
                    emit_attnv(*prev)
                    pending.append((u, qi, outT, aoT))

                    # interleave prior unit's second O-projection (chunk B
                    # completes during this unit's projections)
                    if u == 1 and qi == 1:
                        o_projection(0, 1)

            # tail: last unit's first O-projection before the final
            # collective trigger (cumulative cc semaphore), then qi3's
            # normalize + chunk-B collective, then the last O-projection.
            o_projection(B - 1, 0)
            while pending:
                emit_normalize()
            o_projection(B - 1, 1)

    nc.compile()
    return nc


def _host_inputs(x, wq, wk, wv, wo):
    # x: [B, S, D] -> xh[u, tt, p, dc, t] = x[u, tt*512 + t, dc*128 + p]
    xh = np.ascontiguousarray(
        x.reshape(B, 4, 512, 8, 128).transpose(0, 1, 4, 3, 2)
    ).astype(np.float16)

    p = np.arange(128)
    invf = THETA ** (-2.0 * ((p % 64) // 2) / 64.0)
    ang = invf[:, None] * np.arange(S)[None, :]
    cost = np.cos(ang)
    sinmt = np.sin(ang) * np.where(p % 2 == 0, -1.0, 1.0)[:, None]
    cs = np.stack([cost, sinmt], axis=1).astype(np.float16)  # [128, 2, S]

    i = np.arange(128)[:, None]
    j = np.arange(256)[None, :]
    cb = np.zeros((128, 672), np.float16)
    cb[:, 0:256] = (j >= i).astype(np.float16)          # mask0
    cb[:, 256:512] = (j >= i + 128).astype(np.float16)  # mask1
    cb[:, 512:640] = np.eye(128, dtype=np.float16)      # ident
    cb[:, 640:656] = 1.0                                # ones
    cb[:, 656] = -4.0                                   # exp bias

    woh = np.ascontiguousarray(
        wo.T.reshape(8, 128, D).transpose(1, 0, 2)
    ).astype(np.float16)  # [p, dc, o]

    in_maps = []
    for c in range(NC):
        sl = slice(c * 128, (c + 1) * 128)
        wqkv = np.ascontiguousarray(
            np.stack([wq[sl], wk[sl], wv[sl]]).reshape(3, 128, 8, 128).transpose(3, 0, 2, 1)
        ).astype(np.float16)  # [p, iw, dc, o]
        in_maps.append({
            "xh": xh,
            "wqkv": wqkv,
            "wo": woh,
            "cs": cs,
            "cb": cb,
            "onesr": np.ones((1, 64), np.float32),
        })
    return in_maps


def kernel(x, wq, wk, wv, wo, _trace=False):
    from concourse.bass_utils import run_bass_kernel_spmd

    if "nc" not in _COMPILED:
        _COMPILED["nc"] = _build()
    nc = _COMPILED["nc"]

    in_maps = _host_inputs(
        np.asarray(x, np.float32), np.asarray(wq, np.float32),
        np.asarray(wk, np.float32), np.asarray(wv, np.float32),
        np.asarray(wo, np.float32),
    )
    res = run_bass_kernel_spmd(nc, in_maps, core_ids=list(range(NC)), trace=_trace)
    _COMPILED["last_result"] = res

    y = np.zeros((B, S, D), np.float32)
    for c in range(NC):
        yc = res.results[c]["y"].reshape(B, 4, 64, D)  # [u, qi, j, o]
        for qi in range(4):
            y[:, qi * 512 + c * 64: qi * 512 + (c + 1) * 64, :] = yc[:, qi]
    return y

